# revision 1
# baseline (speedup 1.0000x reference)
"""Trainium2 Bass kernel for nn_CacheAttention (16-head causal MHA, T=2048 B=4 E=1024).

Sharding: 16 heads split across 8 NeuronCores (2 heads / core).  Each core:
  - projects q/k/v with its 128-column slice of wq/wk/wv (E-contraction on
    partitions, activations streamed in head-dim-transposed layout),
  - runs flash-style attention for its 8 (batch, head) pairs in
    transposed-score layout (scores^T = K_tile.T @ Q, softmax sum picked up
    for free via a ones-column appended to V),
  - applies its 128-row slice of wo, producing a partial [B*T, E] output.
Host sums the 8 partials and adds the output bias.

Perf structure: bf16 compute (f32 PSUM), the two heads' K=64 score matmuls
are emitted back-to-back so they run concurrently in disjoint PE row-groups,
score tiles are double-bank [128,1024] so each ACT exp covers two s-tiles,
softmax normalization is reciprocal(DVE) -> partition_broadcast(GpSimd) ->
multiply(DVE).  The additive mask is applied as exp(mask) multiplied into
the probabilities; when the mask is exactly the causal pattern the kernel
skips fully-masked tiles and uses 4 cached diagonal patterns.
"""

import sys

if "/opt/trn_rl_repo" not in sys.path:
    sys.path.insert(0, "/opt/trn_rl_repo")

import numpy as np
import ml_dtypes

import concourse.mybir as mybir
import concourse.tile as tile
from concourse import bacc
from concourse.bass_utils import run_bass_kernel_spmd
from concourse.masks import make_identity

BF16 = ml_dtypes.bfloat16
F32 = mybir.dt.float32
BF = mybir.dt.bfloat16

T, B, E = 2048, 4, 1024
H, D = 16, 64
NCORES = 8
HPC = H // NCORES          # heads per core = 2
DC = HPC * D               # head-dim columns per core = 128
R = B * T                  # rows (b-major: r = b*T + t) = 8192
KT = E // 128              # E contraction tiles = 8
NCH = T // 512             # q chunks per (b,h) pair = 4
NST = T // 128             # s tiles per (b,h) pair = 16
SCALE = float(D) ** -0.5

_CACHE = {}


def _build(causal: bool, reps: int = 1, variant: str = "base"):
    # variant: base | nopack (heads' QK emitted sequentially) | dmaonly |
    #          noattn (attention replaced by memset) | noproj (proj replaced by memset)
    nc = bacc.Bacc("TRN2", target_bir_lowering=False, debug=False, num_devices=NCORES)

    qT_d = nc.dram_tensor("qT", [E, R], BF, kind="ExternalInput")
    kT_d = nc.dram_tensor("kT", [E, R], BF, kind="ExternalInput")
    vT_d = nc.dram_tensor("vT", [E, R], BF, kind="ExternalInput")
    wqT_d = nc.dram_tensor("wqT", [E, DC], BF, kind="ExternalInput")
    wkT_d = nc.dram_tensor("wkT", [E, DC], BF, kind="ExternalInput")
    wvT_d = nc.dram_tensor("wvT", [E, DC], BF, kind="ExternalInput")
    woT_d = nc.dram_tensor("woT", [DC, E], BF, kind="ExternalInput")
    bq_d = nc.dram_tensor("bq", [DC, 1], F32, kind="ExternalInput")
    bk_d = nc.dram_tensor("bk", [DC, 1], F32, kind="ExternalInput")
    bv_d = nc.dram_tensor("bv", [DC, 1], F32, kind="ExternalInput")
    if causal:
        dm_d = nc.dram_tensor("dmask", [4, 128, 512], BF, kind="ExternalInput")
    else:
        em_d = nc.dram_tensor("emaskT", [T, T], BF, kind="ExternalInput")
    out_d = nc.dram_tensor("out", [R, E], BF, kind="ExternalOutput")

    Exp = mybir.ActivationFunctionType.Exp
    add = mybir.AluOpType.add
    mult = mybir.AluOpType.mult

    with tile.TileContext(nc) as tc:
        with (
            tc.tile_pool(name="wp", bufs=1) as wp,
            tc.tile_pool(name="mp", bufs=2) as mp,
            tc.tile_pool(name="ps", bufs=2, space="PSUM") as ps,
        ):
            # ---- constants / weights (persistent) ----
            wq_sb = wp.tile([128, KT, DC], BF, tag="wq")
            wk_sb = wp.tile([128, KT, DC], BF, tag="wk")
            wv_sb = wp.tile([128, KT, DC], BF, tag="wv")
            for w_sb, w_d in ((wq_sb, wqT_d), (wk_sb, wkT_d), (wv_sb, wvT_d)):
                nc.sync.dma_start(w_sb, w_d.ap().rearrange("(k p) d -> p k d", p=128))
            wo_sb = wp.tile([DC, E], BF, tag="wo")
            nc.sync.dma_start(wo_sb, woT_d.ap())
            bq_sb = wp.tile([DC, 1], F32, tag="bq")
            nc.sync.dma_start(bq_sb, bq_d.ap())
            bk_sb = wp.tile([DC, 1], F32, tag="bk")
            nc.sync.dma_start(bk_sb, bk_d.ap())
            bv_sb = wp.tile([DC, 1], F32, tag="bv")
            nc.sync.dma_start(bv_sb, bv_d.ap())
            ident = wp.tile([128, 128], BF, tag="ident")
            make_identity(nc, ident)
            if causal:
                dm_sb = wp.tile([128, 4 * 512], BF, tag="dm")
                nc.sync.dma_start(
                    dm_sb.rearrange("p (j q) -> p j q", q=512),
                    dm_d.ap().rearrange("j p q -> p j q"),
                )

            for b4 in range(B * reps):
                b = b4 % B
                # ---- projections for batch b (rows b*T .. b*T+T) ----
                qT_b = mp.tile([DC, T], BF, tag="qTb")
                kT_b = mp.tile([DC, T], BF, tag="kTb")
                vTt = mp.tile([DC, T], BF, tag="vTt")
                for (src_d, w_sb, bias, scale, dst) in (
                    (qT_d, wq_sb, bq_sb, SCALE, qT_b),
                    (kT_d, wk_sb, bk_sb, 1.0, kT_b),
                    (vT_d, wv_sb, bv_sb, 1.0, vTt),
                ):
                    xin = mp.tile([128, KT, T], BF, tag="xin")
                    # two 2 MiB DMAs per tensor: big transfers amortize the
                    # per-DMA fixed cost on the (FIFO) SP HWDGE ring
                    src_v = src_d.ap().rearrange("(k p) r -> p k r", p=128)
                    for half in range(2):
                        kk = KT // 2
                        nc.sync.dma_start(
                            xin[:, half * kk : (half + 1) * kk, :],
                            src_v[:, half * kk : (half + 1) * kk, b * T : (b + 1) * T],
                        )
                    if variant in ("dmaonly", "noproj"):
                        if variant == "noproj":
                            nc.vector.memset(dst, 0.02)
                        continue
                    # k-outer so the first matmul only waits on one input tile;
                    # two n-columns accumulate in parallel PSUM banks per pass.
                    for g in range(NCH // 2):
                        pps = [
                            ps.tile([128, 512], F32, tag="mm", name=f"pp{i}")
                            for i in range(2)
                        ]
                        for k in range(KT):
                            for i in range(2):
                                n = 2 * g + i
                                nc.tensor.matmul(
                                    pps[i],
                                    w_sb[:, k, :],
                                    xin[:, k, 512 * n : 512 * (n + 1)],
                                    start=(k == 0),
                                    stop=(k == KT - 1),
                                )
                        for i in range(2):
                            n = 2 * g + i
                            # (x + b) * s on DVE, psum f32 -> sbuf bf16
                            nc.vector.tensor_scalar(
                                dst[:, 512 * n : 512 * (n + 1)], pps[i], bias, scale,
                                add, mult,
                            )

                # ---- v^T -> v natural (+ones cols) via PE transpose ----
                # layout per s-tile j: [v_h0(64) | 1 | v_h1(64) | 1] = 130 cols
                v_nat = mp.tile([128, NST * 130], BF, tag="vnat")
                if variant in ("dmaonly", "noproj"):
                    nc.vector.memset(v_nat, 0.01)
                for j in range(NST if variant not in ("dmaonly", "noproj") else 0):
                    pt = ps.tile([128, 128], BF, tag="mm")
                    nc.tensor.transpose(pt, vTt[:, 128 * j : 128 * (j + 1)], ident)
                    for h in range(HPC):
                        nc.any.tensor_copy(
                            v_nat[:, 130 * j + 65 * h : 130 * j + 65 * h + 64],
                            pt[:, 64 * h : 64 * h + 64],
                        )
                vv = v_nat.rearrange("p (r c) -> p r c", c=65)
                nc.vector.memset(vv[:, :, 64], 1.0)

                # ---- attention, both heads interleaved ----
                attnT_b = mp.tile([DC, T], BF, tag="attnTb")
                if variant in ("dmaonly", "noattn"):
                    nc.vector.memset(attnT_b, 0.01)

                def emit_outproj(rg, attnT_b=attnT_b, b=b):
                    # one 1 MiB coalesced store per 4 row-tiles, on the ACT ring
                    o_big = mp.tile([128, 4, E], BF, tag="osb", name="o_big")
                    for r4 in range(4):
                        r = 4 * rg + r4
                        for n in range(E // 512):
                            o_ps = ps.tile([128, 512], F32, tag="mm", name="o_ps")
                            nc.tensor.matmul(
                                o_ps,
                                attnT_b[:, 128 * r : 128 * (r + 1)],
                                wo_sb[:, 512 * n : 512 * (n + 1)],
                                start=True,
                                stop=True,
                            )
                            nc.any.tensor_copy(
                                o_big[:, r4, 512 * n : 512 * (n + 1)], o_ps
                            )
                    nc.scalar.dma_start(
                        out_d.ap()[
                            b * T + 512 * rg : b * T + 512 * (rg + 1), :
                        ].rearrange("(r p) e -> p r e", p=128),
                        o_big,
                    )

                for c in range(NCH if variant not in ("dmaonly", "noattn") else 0):
                    # out-proj of the previous chunk: its inputs are long ready,
                    # so these matmuls never stall the in-order PE stream
                    if c > 0:
                        emit_outproj(c - 1)
                    n_s = 4 * (c + 1) if causal else NST
                    a_ps = [
                        ps.tile([65, 512], F32, tag=f"at{h}", bufs=1, name=f"a_ps{h}")
                        for h in range(HPC)
                    ]
                    for jp in range(n_s // 2):
                        j0 = 2 * jp
                        sc = [
                            ps.tile([128, 1024], F32, tag=f"sc{h}", bufs=1, name=f"sc{h}")
                            for h in range(HPC)
                        ]
                        # QK: emit the two heads back-to-back per s-tile so the
                        # K=64 matmuls pack into disjoint PE row-groups.
                        qk_order = (
                            [(dj, h) for dj in range(2) for h in range(HPC)]
                            if variant != "nopack"
                            else [(dj, h) for h in range(HPC) for dj in range(2)]
                        )
                        for dj, h in qk_order:
                            j = j0 + dj
                            if True:
                                hs = 64 * h
                                nc.tensor.matmul(
                                    sc[h][:, 512 * dj : 512 * (dj + 1)],
                                    kT_b[hs : hs + 64, 128 * j : 128 * (j + 1)],
                                    qT_b[hs : hs + 64, 512 * c : 512 * (c + 1)],
                                    start=True,
                                    stop=True,
                                )
                        em0 = None
                        if not causal:
                            em0 = mp.tile([128, 1024], BF, tag="em", bufs=3)
                            for dj in range(2):
                                nc.sync.dma_start(
                                    em0[:, 512 * dj : 512 * (dj + 1)],
                                    em_d.ap()[
                                        128 * (j0 + dj) : 128 * (j0 + dj + 1),
                                        512 * c : 512 * (c + 1),
                                    ],
                                )
                        elif j0 >= 4 * c:
                            em0 = dm_sb[:, 512 * (j0 - 4 * c) : 512 * (j0 - 4 * c) + 1024]
                        for h in range(HPC):
                            pT = mp.tile([128, 1024], BF, tag="pT", bufs=4)
                            nc.scalar.activation(pT, sc[h], Exp)
                            if em0 is not None:
                                pm = mp.tile([128, 1024], BF, tag="pm", bufs=2)
                                nc.vector.tensor_tensor(pm, pT, em0, mult)
                                pT = pm
                            for dj in range(2):
                                j = j0 + dj
                                nc.tensor.matmul(
                                    a_ps[h],
                                    v_nat[:, 130 * j + 65 * h : 130 * j + 65 * (h + 1)],
                                    pT[:, 512 * dj : 512 * (dj + 1)],
                                    start=(jp == 0 and dj == 0),
                                    stop=(jp == n_s // 2 - 1 and dj == 1),
                                )
                    for h in range(HPC):
                        hs = 64 * h
                        rl = mp.tile([1, 512], BF, tag="rl", bufs=2)
                        with nc.allow_low_precision(reason="softmax denom recip"):
                            nc.vector.reciprocal(rl, a_ps[h][64:65, :])
                        rlb = mp.tile([64, 512], BF, tag="rlb", bufs=2)
                        nc.gpsimd.partition_broadcast(rlb, rl)
                        nc.vector.tensor_tensor(
                            attnT_b[hs : hs + 64, 512 * c : 512 * (c + 1)],
                            a_ps[h][0:64, :],
                            rlb,
                            mult,
                        )

                # ---- output projection (partial; host sums cores) ----
                if variant == "dmaonly":
                    for rg in range(T // 512):
                        o_big = mp.tile([128, 4, E], BF, tag="osb", name="o_big")
                        nc.vector.memset(o_big, 0.0)
                        nc.scalar.dma_start(
                            out_d.ap()[
                                b * T + 512 * rg : b * T + 512 * (rg + 1), :
                            ].rearrange("(r p) e -> p r e", p=128),
                            o_big,
                        )
                elif variant == "noattn":
                    for rg in range(T // 512):
                        emit_outproj(rg)
                else:
                    emit_outproj(T // 512 - 1)


    nc.compile()
    return nc


def _causal_mask_ref():
    return np.where(
        np.arange(T)[:, None] >= np.arange(T)[None, :], np.float32(0.0), np.float32(-1e9)
    ).astype(np.float32)


def _diag_patterns():
    # pattern[j, s, q] = 1.0 if (128*j + s) <= q else 0.0   (q in 0..511)
    j = np.arange(4)[:, None, None]
    s = np.arange(128)[None, :, None]
    q = np.arange(512)[None, None, :]
    return ((128 * j + s) <= q).astype(BF16)


def _prep_in_maps(query, key, value, attn_mask, wq, bq, wk, bk, wv, bv, wo, causal):
    # [T, B, E] -> [E, B*T] b-major columns, bf16
    qT = np.ascontiguousarray(query.transpose(2, 1, 0).reshape(E, R)).astype(BF16)
    kT = np.ascontiguousarray(key.transpose(2, 1, 0).reshape(E, R)).astype(BF16)
    vT = np.ascontiguousarray(value.transpose(2, 1, 0).reshape(E, R)).astype(BF16)
    common = {"qT": qT, "kT": kT, "vT": vT}
    if causal:
        common["dmask"] = np.ascontiguousarray(_diag_patterns())
    else:
        common["emaskT"] = np.exp(attn_mask.astype(np.float64).T).astype(BF16)
    in_maps = []
    for c in range(NCORES):
        sl = slice(DC * c, DC * (c + 1))
        m = dict(common)
        m["wqT"] = np.ascontiguousarray(wq[sl, :].T).astype(BF16)
        m["wkT"] = np.ascontiguousarray(wk[sl, :].T).astype(BF16)
        m["wvT"] = np.ascontiguousarray(wv[sl, :].T).astype(BF16)
        m["woT"] = np.ascontiguousarray(wo[:, sl].T).astype(BF16)
        m["bq"] = bq[sl].astype(np.float32)[:, None]
        m["bk"] = bk[sl].astype(np.float32)[:, None]
        m["bv"] = bv[sl].astype(np.float32)[:, None]
        in_maps.append(m)
    return in_maps


def _postprocess(results, bo):
    acc = results[0]["out"].astype(np.float32)
    for c in range(1, NCORES):
        acc = acc + results[c]["out"].astype(np.float32)
    out = acc.reshape(B, T, E).transpose(1, 0, 2) + bo[None, None, :]
    return np.ascontiguousarray(out.astype(np.float32))


def kernel(query, key, value, attn_mask, wq, bq, wk, bk, wv, bv, wo, bo):
    assert query.shape == (T, B, E), query.shape
    causal = bool(np.array_equal(attn_mask, _causal_mask_ref()))
    if causal not in _CACHE:
        _CACHE[causal] = _build(causal)
    nc = _CACHE[causal]
    in_maps = _prep_in_maps(
        query, key, value, attn_mask, wq, bq, wk, bk, wv, bv, wo, causal
    )
    res = run_bass_kernel_spmd(nc, in_maps, core_ids=list(range(NCORES)))
    return _postprocess(res.results, np.asarray(bo, dtype=np.float32))



# revision 7
# speedup vs baseline: 1.0275x; 1.0275x over previous
"""Trainium2 Bass kernel for nn_CacheAttention (16-head causal MHA, T=2048 B=4 E=1024).

Sharding v2: core = (batch, head-half).  8 cores = 4 batches x 2 halves; each
core owns ONE batch and 8 heads (512 projection columns, processed as 4
partition-groups of 128 rows = 2 heads each).  vs v1 (2 heads x 4 batches per
core) this cuts per-core HBM traffic from 64 MB to 20 MB: q/k/v activations
for the core's batch are loaded once per rep and reused by all 4 groups.

Perf structure: the attention inner loop is ACT(exp)-bound, and the PE queue
is in-order, so every non-attention matmul (projections of the NEXT
partition-group, out-projection once the last group's chunk is normalized) is
emitted as fine-grained "feed" thunks BETWEEN attention instructions.  This
keeps the PE stream continuously busy, which both hides the work and holds
the PE at its top p-state (TRN2 DVFS: 2.4 GHz only after 3 us of
uninterrupted execution).  PV matmuls run one jp behind their exp (softmax
probabilities) so the PE never waits on ACT.  Score tiles are [128,1024]
PSUM (two s-tiles per exp); softmax sums ride along as a ones-column in V;
the causal path skips fully-masked tiles and uses 4 cached diagonal patterns.

Host sums each batch's two partial outputs and adds the output bias.
"""

import sys

if "/opt/trn_rl_repo" not in sys.path:
    sys.path.insert(0, "/opt/trn_rl_repo")

from collections import deque

import numpy as np
import ml_dtypes

import concourse.mybir as mybir
import concourse.tile as tile
from concourse import bacc
from concourse.bass_utils import run_bass_kernel_spmd
from concourse.masks import make_identity

BF16 = ml_dtypes.bfloat16
F32 = mybir.dt.float32
BF = mybir.dt.bfloat16

T, B, E = 2048, 4, 1024
H, D = 16, 64
NCORES = 8
HPC = 2                    # heads per partition-group
DC = 512                   # head-dim columns per core (8 heads)
NPG = DC // 128            # partition groups per core = 4
KT = E // 128              # E contraction tiles = 8
NCH = T // 512             # q chunks per (b,h) pair = 4
NST = T // 128             # s tiles = 16
SCALE = float(D) ** -0.5

_CACHE = {}


def _build(causal: bool, reps: int = 1, variant: str = "base"):
    # variant: base | dmaonly | noattn (attention replaced by memset) |
    #          noproj (projections replaced by memset) | nofeed (no
    #          interleaving: proj/outproj emitted flat)
    nc = bacc.Bacc("TRN2", target_bir_lowering=False, debug=False, num_devices=NCORES)

    xq_d = nc.dram_tensor("xq", [E, T], BF, kind="ExternalInput")
    xk_d = nc.dram_tensor("xk", [E, T], BF, kind="ExternalInput")
    xv_d = nc.dram_tensor("xv", [E, T], BF, kind="ExternalInput")
    wqT_d = nc.dram_tensor("wqT", [E, DC], BF, kind="ExternalInput")
    wkT_d = nc.dram_tensor("wkT", [E, DC], BF, kind="ExternalInput")
    wvT_d = nc.dram_tensor("wvT", [E, DC], BF, kind="ExternalInput")
    woT_d = nc.dram_tensor("woT", [DC, E], BF, kind="ExternalInput")
    bq_d = nc.dram_tensor("bq", [128, NPG], F32, kind="ExternalInput")
    bk_d = nc.dram_tensor("bk", [128, NPG], F32, kind="ExternalInput")
    bv_d = nc.dram_tensor("bv", [128, NPG], F32, kind="ExternalInput")
    if causal:
        dm_d = nc.dram_tensor("dmask", [4, 128, 512], BF, kind="ExternalInput")
    else:
        em_d = nc.dram_tensor("emaskT", [T, T], BF, kind="ExternalInput")
    out_d = nc.dram_tensor("out", [T, E], BF, kind="ExternalOutput")

    Exp = mybir.ActivationFunctionType.Exp
    add = mybir.AluOpType.add
    mult = mybir.AluOpType.mult

    with tile.TileContext(nc) as tc:
        with (
            tc.tile_pool(name="wp", bufs=1) as wp,
            tc.tile_pool(name="mp", bufs=1) as mp,
            tc.tile_pool(name="ps", bufs=2, space="PSUM") as ps,
        ):
            # ---- constants / weights (persistent) ----
            wq_sb = wp.tile([128, KT, DC], BF, tag="wq")
            wk_sb = wp.tile([128, KT, DC], BF, tag="wk")
            wv_sb = wp.tile([128, KT, DC], BF, tag="wv")
            for w_sb, w_d in ((wq_sb, wqT_d), (wk_sb, wkT_d), (wv_sb, wvT_d)):
                nc.sync.dma_start(w_sb, w_d.ap().rearrange("(k p) d -> p k d", p=128))
            wo_sb = wp.tile([128, NPG, E], BF, tag="wo")
            nc.sync.dma_start(wo_sb, woT_d.ap().rearrange("(g p) e -> p g e", p=128))
            bq_sb = wp.tile([128, NPG], F32, tag="bq")
            nc.sync.dma_start(bq_sb, bq_d.ap())
            bk_sb = wp.tile([128, NPG], F32, tag="bk")
            nc.sync.dma_start(bk_sb, bk_d.ap())
            bv_sb = wp.tile([128, NPG], F32, tag="bv")
            nc.sync.dma_start(bv_sb, bv_d.ap())
            ident = wp.tile([128, 128], BF, tag="ident")
            make_identity(nc, ident)
            if causal:
                dm_sb = wp.tile([128, 4 * 512], BF, tag="dm")
                nc.sync.dma_start(
                    dm_sb.rearrange("p (j q) -> p j q", q=512),
                    dm_d.ap().rearrange("j p q -> p j q"),
                )

            # rep-persistent activation tiles
            qT_all = wp.tile([128, NPG, T], BF, tag="qT")
            kT_all = wp.tile([128, NPG, T], BF, tag="kT")
            v_nat = wp.tile([128, NPG, NST * 130], BF, tag="vnat")
            attnT = wp.tile([128, NPG, T], BF, tag="attnT")

            def proj_thunks(pg):
                """Feed thunks computing qT/kT/v_nat for group pg: 3 x
                (dma + 2 psum-pair passes over 8 k-tiles + bias) + v
                transposes.  Each thunk is a small, stall-free PE burst."""
                thunks = []
                plan = (
                    (xq_d, wq_sb, bq_sb, SCALE, qT_all),
                    (xk_d, wk_sb, bk_sb, 1.0, kT_all),
                    (xv_d, wv_sb, bv_sb, 1.0, None),
                )
                xins = {}
                vTt = None

                def dma_xin(i):
                    def th():
                        src_d = plan[i][0]
                        xt = mp.tile([128, KT, T], BF, tag="xin", bufs=2, name="xin")
                        src_v = src_d.ap().rearrange("(k p) r -> p k r", p=128)
                        kk = KT // 2
                        for half in range(2):
                            nc.sync.dma_start(
                                xt[:, half * kk : (half + 1) * kk, :],
                                src_v[:, half * kk : (half + 1) * kk, :],
                            )
                        xins[i] = xt
                    return th

                def mk_group(i, g):
                    # one psum pair accumulating two 512-col chunks over all k
                    st = {}

                    def alloc():
                        st["pps"] = [
                            ps.tile([128, 512], F32, tag="mm", name=f"pp{j}")
                            for j in range(2)
                        ]

                    def mm(k):
                        def th():
                            if "pps" not in st:
                                alloc()
                            w_sb = plan[i][1]
                            for j in range(2):
                                n = 2 * g + j
                                nc.tensor.matmul(
                                    st["pps"][j],
                                    w_sb[:, k, 128 * pg : 128 * (pg + 1)],
                                    xins[i][:, k, 512 * n : 512 * (n + 1)],
                                    start=(k == 0),
                                    stop=(k == KT - 1),
                                )
                        return th

                    def bias():
                        nonlocal vTt
                        _, _, b_sb, scale, dst = plan[i]
                        if dst is None and vTt is None:
                            vTt = mp.tile([128, T], BF, tag="vTt", bufs=1)
                        for j in range(2):
                            n = 2 * g + j
                            dst_ap = (
                                vTt[:, 512 * n : 512 * (n + 1)]
                                if dst is None
                                else dst[:, pg, 512 * n : 512 * (n + 1)]
                            )
                            nc.vector.tensor_scalar(
                                dst_ap,
                                st["pps"][j],
                                b_sb[:, pg : pg + 1],
                                scale,
                                add,
                                mult,
                            )
                    return [mm(k) for k in range(KT)] + [bias]

                thunks.append(dma_xin(0))
                thunks.append(dma_xin(1))
                thunks.extend(mk_group(0, 0))
                thunks.extend(mk_group(0, 1))
                thunks.extend(mk_group(1, 0))
                thunks.append(dma_xin(2))
                thunks.extend(mk_group(1, 1))
                thunks.extend(mk_group(2, 0))
                thunks.extend(mk_group(2, 1))

                def ones():
                    vv = v_nat[:, pg, :].rearrange("p (r c) -> p r c", c=65)
                    nc.vector.memset(vv[:, :, 64], 1.0)
                thunks.append(ones)

                def transp(j):
                    def th():
                        pt = ps.tile([128, 128], BF, tag="mm", name="pt")
                        nc.tensor.transpose(pt, vTt[:, 128 * j : 128 * (j + 1)], ident)
                        for h in range(HPC):
                            nc.any.tensor_copy(
                                v_nat[:, pg, 130 * j + 65 * h : 130 * j + 65 * h + 64],
                                pt[:, 64 * h : 64 * h + 64],
                            )
                    return th

                thunks.extend(transp(j) for j in range(NST))
                return thunks

            def outproj_chunk_thunks(c, st):
                # out rows 512c..512c+511; contraction over all 4 groups
                thunks = []

                def alloc():
                    st["o_big"] = mp.tile([128, 4, E], BF, tag="osb", bufs=2, name="o_big")

                def mk(r4, n):
                    def th():
                        if "o_big" not in st:
                            alloc()
                        r = 4 * c + r4
                        o_ps = ps.tile([128, 512], F32, tag="mm", name="o_ps")
                        for pg in range(NPG):
                            nc.tensor.matmul(
                                o_ps,
                                attnT[:, pg, 128 * r : 128 * (r + 1)],
                                wo_sb[:, pg, 512 * n : 512 * (n + 1)],
                                start=(pg == 0),
                                stop=(pg == NPG - 1),
                            )
                        nc.any.tensor_copy(o_big_slice(st, r4, n), o_ps)
                    return th

                def o_big_slice(st, r4, n):
                    return st["o_big"][:, r4, 512 * n : 512 * (n + 1)]

                for r4 in range(4):
                    for n in range(E // 512):
                        thunks.append(mk(r4, n))

                def store():
                    nc.gpsimd.dma_start(
                        out_d.ap()[512 * c : 512 * (c + 1), :].rearrange(
                            "(r p) e -> p r e", p=128
                        ),
                        st.pop("o_big"),
                    )
                thunks.append(store)
                return thunks

            def attention(pg, feeds, on_chunk_done=None):
                """Attention for group pg (2 heads), feeding `feeds` thunks
                into the PE stream.  PV runs one jp behind its exp.
                on_chunk_done(c) may extend `feeds` (out-proj release)."""
                def feed(k):
                    for _ in range(k):
                        if feeds:
                            feeds.popleft()()

                for c in range(NCH):
                    n_s = 4 * (c + 1) if causal else NST
                    a_ps = [
                        ps.tile([65, 512], F32, tag=f"at{h}", bufs=1, name=f"a_ps{h}")
                        for h in range(HPC)
                    ]
                    prev = None

                    def emit_pv(item, n_s=n_s):
                        pTs, jpp = item
                        for h in range(HPC):
                            for dj in range(2):
                                nc.tensor.matmul(
                                    a_ps[h],
                                    v_nat[
                                        :,
                                        pg,
                                        130 * (2 * jpp + dj) + 65 * h : 130 * (2 * jpp + dj) + 65 * (h + 1),
                                    ],
                                    pTs[h][:, 512 * dj : 512 * (dj + 1)],
                                    start=(jpp == 0 and dj == 0),
                                    stop=(jpp == n_s // 2 - 1 and dj == 1),
                                )

                    for jp in range(n_s // 2):
                        j0 = 2 * jp
                        sc = [
                            ps.tile([128, 1024], F32, tag=f"sc{h}", bufs=1, name=f"sc{h}")
                            for h in range(HPC)
                        ]
                        for h in range(HPC):
                            hs = 64 * h
                            for dj in range(2):
                                j = j0 + dj
                                nc.tensor.matmul(
                                    sc[h][:, 512 * dj : 512 * (dj + 1)],
                                    kT_all[hs : hs + 64, pg, 128 * j : 128 * (j + 1)],
                                    qT_all[hs : hs + 64, pg, 512 * c : 512 * (c + 1)],
                                    start=True,
                                    stop=True,
                                )
                        feed(2)
                        em0 = None
                        if not causal:
                            em0 = mp.tile([128, 1024], BF, tag="em", bufs=4, name="em")
                            for dj in range(2):
                                nc.sync.dma_start(
                                    em0[:, 512 * dj : 512 * (dj + 1)],
                                    em_d.ap()[
                                        128 * (j0 + dj) : 128 * (j0 + dj + 1),
                                        512 * c : 512 * (c + 1),
                                    ],
                                )
                        elif j0 >= 4 * c:
                            em0 = dm_sb[:, 512 * (j0 - 4 * c) : 512 * (j0 - 4 * c) + 1024]
                        pTs = []
                        for h in range(HPC):
                            pT = mp.tile([128, 1024], BF, tag="pT", bufs=4, name="pT")
                            nc.scalar.activation(pT, sc[h], Exp)
                            if em0 is not None:
                                pm = mp.tile([128, 1024], BF, tag="pm", bufs=4, name="pm")
                                nc.vector.tensor_tensor(pm, pT, em0, mult)
                                pT = pm
                            pTs.append(pT)
                        if prev is not None:
                            emit_pv(prev)
                        feed(2)
                        prev = (pTs, jp)
                    if prev is not None:
                        emit_pv(prev)
                    # softmax normalization -> attnT
                    for h in range(HPC):
                        hs = 64 * h
                        rl = mp.tile([1, 512], BF, tag="rl", bufs=2, name="rl")
                        with nc.allow_low_precision(reason="softmax denom recip"):
                            nc.vector.reciprocal(rl, a_ps[h][64:65, :])
                        rlb = mp.tile([64, 512], BF, tag="rlb", bufs=2, name="rlb")
                        nc.gpsimd.partition_broadcast(rlb, rl)
                        nc.vector.tensor_tensor(
                            attnT[hs : hs + 64, pg, 512 * c : 512 * (c + 1)],
                            a_ps[h][0:64, :],
                            rlb,
                            mult,
                        )
                    if on_chunk_done is not None:
                        on_chunk_done(c)

            def memset_group(pg):
                nc.vector.memset(qT_all[:, pg, :], 0.02)
                nc.vector.memset(kT_all[:, pg, :], 0.02)
                nc.vector.memset(v_nat[:, pg, :], 0.01)

            # ================= main schedule =================
            for rep in range(reps):
                if variant in ("dmaonly", "noproj"):
                    # still move the input bytes
                    for src_d in (xq_d, xk_d, xv_d):
                        xt = mp.tile([128, KT, T], BF, tag="xin", bufs=2, name="xin")
                        nc.sync.dma_start(
                            xt, src_d.ap().rearrange("(k p) r -> p k r", p=128)
                        )
                    for pg in range(NPG):
                        memset_group(pg)
                else:
                    for th in proj_thunks(0):
                        th()

                if variant in ("dmaonly", "noattn"):
                    for pg in range(NPG):
                        nc.vector.memset(attnT[:, pg, :], 0.01)
                    if variant == "noattn":
                        for pg in range(1, NPG):
                            for th in proj_thunks(pg):
                                th()
                        st = {}
                        for c in range(NCH):
                            for th in outproj_chunk_thunks(c, st):
                                th()
                    else:
                        for c in range(NCH):
                            o_big = mp.tile(
                                [128, 4, E], BF, tag="osb", bufs=2, name="o_big"
                            )
                            nc.vector.memset(o_big, 0.0)
                            nc.gpsimd.dma_start(
                                out_d.ap()[512 * c : 512 * (c + 1), :].rearrange(
                                    "(r p) e -> p r e", p=128
                                ),
                                o_big,
                            )
                else:
                    for pg in range(NPG):
                        if pg < NPG - 1 and variant != "noproj":
                            feeds = deque(proj_thunks(pg + 1))
                        else:
                            feeds = deque()
                        if pg == NPG - 1:
                            op_state = {}

                            def release(c, feeds=feeds, op_state=op_state):
                                feeds.extend(outproj_chunk_thunks(c, op_state))

                            attention(pg, feeds, on_chunk_done=release)
                        else:
                            attention(pg, feeds)
                        while feeds:
                            feeds.popleft()()

    nc.compile()
    return nc


def _causal_mask_ref():
    return np.where(
        np.arange(T)[:, None] >= np.arange(T)[None, :], np.float32(0.0), np.float32(-1e9)
    ).astype(np.float32)


def _diag_patterns():
    # pattern[j, s, q] = 1.0 if (128*j + s) <= q else 0.0   (q in 0..511)
    j = np.arange(4)[:, None, None]
    s = np.arange(128)[None, :, None]
    q = np.arange(512)[None, None, :]
    return ((128 * j + s) <= q).astype(BF16)


def _prep_in_maps(query, key, value, attn_mask, wq, bq, wk, bk, wv, bv, wo, causal):
    common = {}
    if causal:
        common["dmask"] = np.ascontiguousarray(_diag_patterns())
    else:
        common["emaskT"] = np.exp(attn_mask.astype(np.float64).T).astype(BF16)
    in_maps = []
    xT = {}
    for b in range(B):
        xT[b] = {
            "xq": np.ascontiguousarray(query[:, b, :].T).astype(BF16),
            "xk": np.ascontiguousarray(key[:, b, :].T).astype(BF16),
            "xv": np.ascontiguousarray(value[:, b, :].T).astype(BF16),
        }
    for c in range(NCORES):
        b, hh = divmod(c, 2)
        sl = slice(DC * hh, DC * (hh + 1))
        m = dict(common)
        m.update(xT[b])
        m["wqT"] = np.ascontiguousarray(wq[sl, :].T).astype(BF16)
        m["wkT"] = np.ascontiguousarray(wk[sl, :].T).astype(BF16)
        m["wvT"] = np.ascontiguousarray(wv[sl, :].T).astype(BF16)
        m["woT"] = np.ascontiguousarray(wo[:, sl].T).astype(BF16)
        m["bq"] = np.ascontiguousarray(bq[sl].astype(np.float32).reshape(NPG, 128).T)
        m["bk"] = np.ascontiguousarray(bk[sl].astype(np.float32).reshape(NPG, 128).T)
        m["bv"] = np.ascontiguousarray(bv[sl].astype(np.float32).reshape(NPG, 128).T)
        in_maps.append(m)
    return in_maps


def _postprocess(results, bo):
    outs = []
    for b in range(B):
        acc = results[2 * b]["out"].astype(np.float32) + results[2 * b + 1][
            "out"
        ].astype(np.float32)
        outs.append(acc)
    out = np.stack(outs, axis=1) + bo[None, None, :]
    return np.ascontiguousarray(out.astype(np.float32))


def kernel(query, key, value, attn_mask, wq, bq, wk, bk, wv, bv, wo, bo):
    assert query.shape == (T, B, E), query.shape
    causal = bool(np.array_equal(attn_mask, _causal_mask_ref()))
    if causal not in _CACHE:
        _CACHE[causal] = _build(causal)
    nc = _CACHE[causal]
    in_maps = _prep_in_maps(
        query, key, value, attn_mask, wq, bq, wk, bk, wv, bv, wo, causal
    )
    res = run_bass_kernel_spmd(nc, in_maps, core_ids=list(range(NCORES)))
    return _postprocess(res.results, np.asarray(bo, dtype=np.float32))


# revision 12
# speedup vs baseline: 1.0813x; 1.0524x over previous
"""Trainium2 Bass kernel for nn_CacheAttention (16-head causal MHA, T=2048 B=4 E=1024).

Sharding v2: core = (batch, head-half).  8 cores = 4 batches x 2 halves; each
core owns ONE batch and 8 heads (512 projection columns, processed as 4
partition-groups of 128 rows = 2 heads each).  vs v1 (2 heads x 4 batches per
core) this cuts per-core HBM traffic from 64 MB to 20 MB: q/k/v activations
for the core's batch are loaded once per rep and reused by all 4 groups.

Perf structure: the attention inner loop is ACT(exp)-bound, and the PE queue
is in-order, so every non-attention matmul (projections of the NEXT
partition-group, out-projection once the last group's chunk is normalized) is
emitted as fine-grained "feed" thunks BETWEEN attention instructions.  This
keeps the PE stream continuously busy, which both hides the work and holds
the PE at its top p-state (TRN2 DVFS: 2.4 GHz only after 3 us of
uninterrupted execution).  PV matmuls run one jp behind their exp (softmax
probabilities) so the PE never waits on ACT.  Score tiles are [128,1024]
PSUM (two s-tiles per exp); softmax sums ride along as a ones-column in V;
the causal path skips fully-masked tiles and uses 4 cached diagonal patterns.

Host sums each batch's two partial outputs and adds the output bias.
"""

import sys

if "/opt/trn_rl_repo" not in sys.path:
    sys.path.insert(0, "/opt/trn_rl_repo")

from collections import deque

import numpy as np
import ml_dtypes

import concourse.mybir as mybir
import concourse.tile as tile
from concourse import bacc
from concourse.bass_utils import run_bass_kernel_spmd
from concourse.masks import make_identity

BF16 = ml_dtypes.bfloat16
F32 = mybir.dt.float32
BF = mybir.dt.bfloat16

T, B, E = 2048, 4, 1024
H, D = 16, 64
NCORES = 8
HPC = 2                    # heads per partition-group
DC = 512                   # head-dim columns per core (8 heads)
NPG = DC // 128            # partition groups per core = 4
KT = E // 128              # E contraction tiles = 8
NCH = T // 512             # q chunks per (b,h) pair = 4
NST = T // 128             # s tiles = 16
SCALE = float(D) ** -0.5

_CACHE = {}


def _build(causal: bool, reps: int = 1, variant: str = "base"):
    # variant: base | dmaonly | noattn (attention replaced by memset) |
    #          noproj (projections replaced by memset) | nofeed (no
    #          interleaving: proj/outproj emitted flat)
    nc = bacc.Bacc("TRN2", target_bir_lowering=False, debug=False, num_devices=NCORES)

    xq_d = nc.dram_tensor("xq", [E, T], BF, kind="ExternalInput")
    xk_d = nc.dram_tensor("xk", [E, T], BF, kind="ExternalInput")
    xv_d = nc.dram_tensor("xv", [E, T], BF, kind="ExternalInput")
    wqT_d = nc.dram_tensor("wqT", [E, DC], BF, kind="ExternalInput")
    wkT_d = nc.dram_tensor("wkT", [E, DC], BF, kind="ExternalInput")
    wvT_d = nc.dram_tensor("wvT", [E, DC], BF, kind="ExternalInput")
    woT_d = nc.dram_tensor("woT", [DC, E], BF, kind="ExternalInput")
    bq_d = nc.dram_tensor("bq", [128, NPG], F32, kind="ExternalInput")
    bk_d = nc.dram_tensor("bk", [128, NPG], F32, kind="ExternalInput")
    bv_d = nc.dram_tensor("bv", [128, NPG], F32, kind="ExternalInput")
    if causal:
        dm_d = nc.dram_tensor("dmask", [4, 128, 512], BF, kind="ExternalInput")
    else:
        em_d = nc.dram_tensor("emaskT", [T, T], BF, kind="ExternalInput")
    out_d = nc.dram_tensor("out", [T, E], BF, kind="ExternalOutput")

    Exp = mybir.ActivationFunctionType.Exp
    add = mybir.AluOpType.add
    mult = mybir.AluOpType.mult

    with tile.TileContext(nc) as tc:
        with (
            tc.tile_pool(name="wp", bufs=1) as wp,
            tc.tile_pool(name="mp", bufs=1) as mp,
            tc.tile_pool(name="ps", bufs=2, space="PSUM") as ps,
        ):
            # ---- constants / weights (persistent) ----
            wq_sb = wp.tile([128, KT, DC], BF, tag="wq")
            wk_sb = wp.tile([128, KT, DC], BF, tag="wk")
            wv_sb = wp.tile([128, KT, DC], BF, tag="wv")
            for w_sb, w_d in ((wq_sb, wqT_d), (wk_sb, wkT_d), (wv_sb, wvT_d)):
                nc.sync.dma_start(w_sb, w_d.ap().rearrange("(k p) d -> p k d", p=128))
            wo_sb = wp.tile([128, NPG, E], BF, tag="wo")
            nc.sync.dma_start(wo_sb, woT_d.ap().rearrange("(g p) e -> p g e", p=128))
            bq_sb = wp.tile([128, NPG], F32, tag="bq")
            nc.sync.dma_start(bq_sb, bq_d.ap())
            bk_sb = wp.tile([128, NPG], F32, tag="bk")
            nc.sync.dma_start(bk_sb, bk_d.ap())
            bv_sb = wp.tile([128, NPG], F32, tag="bv")
            nc.sync.dma_start(bv_sb, bv_d.ap())
            ident = wp.tile([128, 128], BF, tag="ident")
            make_identity(nc, ident)
            if causal:
                dm_sb = wp.tile([128, 4 * 512], BF, tag="dm")
                nc.sync.dma_start(
                    dm_sb.rearrange("p (j q) -> p j q", q=512),
                    dm_d.ap().rearrange("j p q -> p j q"),
                )

            # rep-persistent activation tiles
            qT_all = wp.tile([128, NPG, T], BF, tag="qT")
            kT_all = wp.tile([128, NPG, T], BF, tag="kT")
            v_nat = wp.tile([128, NPG, NST * 130], BF, tag="vnat")
            attnT = wp.tile([128, NPG, T], BF, tag="attnT")

            def proj_thunks(pg):
                """Feed thunks computing qT/kT/v_nat for group pg: 3 x
                (dma + 2 psum-pair passes over 8 k-tiles + bias) + v
                transposes.  Each thunk is a small, stall-free PE burst."""
                thunks = []
                plan = (
                    (xq_d, wq_sb, bq_sb, SCALE, qT_all),
                    (xk_d, wk_sb, bk_sb, 1.0, kT_all),
                    (xv_d, wv_sb, bv_sb, 1.0, None),
                )
                xins = {}
                vTt = None

                def dma_xin(i):
                    def th():
                        src_d = plan[i][0]
                        xt = mp.tile([128, KT, T], BF, tag="xin", bufs=2, name="xin")
                        src_v = src_d.ap().rearrange("(k p) r -> p k r", p=128)
                        kk = KT // 2
                        for half in range(2):
                            nc.sync.dma_start(
                                xt[:, half * kk : (half + 1) * kk, :],
                                src_v[:, half * kk : (half + 1) * kk, :],
                            )
                        xins[i] = xt
                    return th

                def mk_group(i, g):
                    # one psum pair accumulating two 512-col chunks over all k
                    st = {}

                    def alloc():
                        st["pps"] = [
                            ps.tile([128, 512], F32, tag="mm", name=f"pp{j}")
                            for j in range(2)
                        ]

                    def mm(k):
                        def th():
                            if "pps" not in st:
                                alloc()
                            w_sb = plan[i][1]
                            for j in range(2):
                                n = 2 * g + j
                                nc.tensor.matmul(
                                    st["pps"][j],
                                    w_sb[:, k, 128 * pg : 128 * (pg + 1)],
                                    xins[i][:, k, 512 * n : 512 * (n + 1)],
                                    start=(k == 0),
                                    stop=(k == KT - 1),
                                )
                        return th

                    def bias():
                        nonlocal vTt
                        _, _, b_sb, scale, dst = plan[i]
                        if dst is None and vTt is None:
                            vTt = mp.tile([128, T], BF, tag="vTt", bufs=1)
                        for j in range(2):
                            n = 2 * g + j
                            dst_ap = (
                                vTt[:, 512 * n : 512 * (n + 1)]
                                if dst is None
                                else dst[:, pg, 512 * n : 512 * (n + 1)]
                            )
                            nc.vector.tensor_scalar(
                                dst_ap,
                                st["pps"][j],
                                b_sb[:, pg : pg + 1],
                                scale,
                                add,
                                mult,
                            )
                    return [mm(k) for k in range(KT)] + [bias]

                thunks.append(dma_xin(0))
                thunks.append(dma_xin(1))
                thunks.extend(mk_group(0, 0))
                thunks.extend(mk_group(0, 1))
                thunks.extend(mk_group(1, 0))
                thunks.append(dma_xin(2))
                thunks.extend(mk_group(1, 1))
                thunks.extend(mk_group(2, 0))
                thunks.extend(mk_group(2, 1))

                def ones():
                    vv = v_nat[:, pg, :].rearrange("p (r c) -> p r c", c=65)
                    nc.vector.memset(vv[:, :, 64], 1.0)
                thunks.append(ones)

                def transp(j):
                    def th():
                        pt = ps.tile([128, 128], BF, tag="mm", name="pt")
                        nc.tensor.transpose(pt, vTt[:, 128 * j : 128 * (j + 1)], ident)
                        for h in range(HPC):
                            nc.vector.tensor_copy(
                                v_nat[:, pg, 130 * j + 65 * h : 130 * j + 65 * h + 64],
                                pt[:, 64 * h : 64 * h + 64],
                            )
                    return th

                thunks.extend(transp(j) for j in range(NST))
                return thunks

            def outproj_chunk_thunks(c, st):
                # out rows 512c..512c+511; contraction over all 4 groups
                thunks = []

                def alloc():
                    st["o_big"] = mp.tile([128, 4, E], BF, tag="osb", bufs=2, name="o_big")

                def mk(r4, n):
                    def th():
                        if "o_big" not in st:
                            alloc()
                        r = 4 * c + r4
                        o_ps = ps.tile([128, 512], F32, tag="mm", name="o_ps")
                        for pg in range(NPG):
                            nc.tensor.matmul(
                                o_ps,
                                attnT[:, pg, 128 * r : 128 * (r + 1)],
                                wo_sb[:, pg, 512 * n : 512 * (n + 1)],
                                start=(pg == 0),
                                stop=(pg == NPG - 1),
                            )
                        nc.vector.tensor_copy(o_big_slice(st, r4, n), o_ps)
                    return th

                def o_big_slice(st, r4, n):
                    return st["o_big"][:, r4, 512 * n : 512 * (n + 1)]

                for r4 in range(4):
                    for n in range(E // 512):
                        thunks.append(mk(r4, n))

                def store():
                    nc.gpsimd.dma_start(
                        out_d.ap()[512 * c : 512 * (c + 1), :].rearrange(
                            "(r p) e -> p r e", p=128
                        ),
                        st.pop("o_big"),
                    )
                thunks.append(store)
                return thunks

            def attention(pg, feeds, on_chunk_done=None):
                """Attention for group pg (2 heads), feeding `feeds` thunks
                into the PE stream.  PV runs one jp behind its exp.
                on_chunk_done(c) may extend `feeds` (out-proj release)."""
                def feed(k):
                    for _ in range(k):
                        if feeds:
                            feeds.popleft()()

                for c in range(NCH):
                    n_s = 4 * (c + 1) if causal else NST
                    a_ps = [
                        ps.tile([65, 512], F32, tag=f"at{h}", bufs=1, name=f"a_ps{h}")
                        for h in range(HPC)
                    ]
                    prev = None

                    def emit_pv(item, n_s=n_s):
                        # pTs keyed (h, dj); each [128, 512]
                        pTs, jpp = item
                        for h in range(HPC):
                            for dj in range(2):
                                nc.tensor.matmul(
                                    a_ps[h],
                                    v_nat[
                                        :,
                                        pg,
                                        130 * (2 * jpp + dj) + 65 * h : 130 * (2 * jpp + dj) + 65 * (h + 1),
                                    ],
                                    pTs[h][dj],
                                    start=(jpp == 0 and dj == 0),
                                    stop=(jpp == n_s // 2 - 1 and dj == 1),
                                )

                    for jp in range(n_s // 2):
                        j0 = 2 * jp
                        # one sc tag, bufs=4, allocated in the SAME (dj-major)
                        # order the ACTs free them: next jp's first QK pair
                        # only waits for this jp's first exp, hiding the
                        # PE<->ACT semaphore round-trip entirely.
                        sc = {}
                        for dj in range(2):
                            for h in range(HPC):
                                sc[(h, dj)] = ps.tile(
                                    [128, 512], F32, tag="sc", bufs=4, name="sc"
                                )
                        for dj in range(2):
                            j = j0 + dj
                            for h in range(HPC):
                                hs = 64 * h
                                nc.tensor.matmul(
                                    sc[(h, dj)],
                                    kT_all[hs : hs + 64, pg, 128 * j : 128 * (j + 1)],
                                    qT_all[hs : hs + 64, pg, 512 * c : 512 * (c + 1)],
                                    start=True,
                                    stop=True,
                                )
                        feed(1)
                        em0 = None
                        if not causal:
                            em0 = mp.tile([128, 1024], BF, tag="em", bufs=4, name="em")
                            for dj in range(2):
                                nc.sync.dma_start(
                                    em0[:, 512 * dj : 512 * (dj + 1)],
                                    em_d.ap()[
                                        128 * (j0 + dj) : 128 * (j0 + dj + 1),
                                        512 * c : 512 * (c + 1),
                                    ],
                                )
                        elif j0 >= 4 * c:
                            em0 = dm_sb[:, 512 * (j0 - 4 * c) : 512 * (j0 - 4 * c) + 1024]
                        pTs = {}
                        for dj in range(2):
                            for h in range(HPC):
                                pT = mp.tile([128, 512], BF, tag="pT", bufs=8, name="pT")
                                nc.scalar.activation(pT, sc[(h, dj)], Exp)
                                if em0 is not None:
                                    pm = mp.tile(
                                        [128, 512], BF, tag="pm", bufs=4, name="pm"
                                    )
                                    nc.vector.tensor_tensor(
                                        pm, pT, em0[:, 512 * dj : 512 * (dj + 1)], mult
                                    )
                                    pT = pm
                                pTs.setdefault(h, {})[dj] = pT
                        if prev is not None:
                            emit_pv(prev)
                        feed(2)
                        prev = (pTs, jp)
                    if prev is not None:
                        emit_pv(prev)
                    # softmax normalization -> attnT
                    for h in range(HPC):
                        hs = 64 * h
                        rl = mp.tile([1, 512], BF, tag="rl", bufs=2, name="rl")
                        with nc.allow_low_precision(reason="softmax denom recip"):
                            nc.vector.reciprocal(rl, a_ps[h][64:65, :])
                        rlb = mp.tile([64, 512], BF, tag="rlb", bufs=2, name="rlb")
                        nc.gpsimd.partition_broadcast(rlb, rl)
                        nc.vector.tensor_tensor(
                            attnT[hs : hs + 64, pg, 512 * c : 512 * (c + 1)],
                            a_ps[h][0:64, :],
                            rlb,
                            mult,
                        )
                    if on_chunk_done is not None:
                        on_chunk_done(c)

            def memset_group(pg):
                nc.vector.memset(qT_all[:, pg, :], 0.02)
                nc.vector.memset(kT_all[:, pg, :], 0.02)
                nc.vector.memset(v_nat[:, pg, :], 0.01)

            # ================= main schedule =================
            for rep in range(reps):
                if variant in ("dmaonly", "noproj"):
                    # still move the input bytes
                    for src_d in (xq_d, xk_d, xv_d):
                        xt = mp.tile([128, KT, T], BF, tag="xin", bufs=2, name="xin")
                        nc.sync.dma_start(
                            xt, src_d.ap().rearrange("(k p) r -> p k r", p=128)
                        )
                    for pg in range(NPG):
                        memset_group(pg)
                else:
                    for th in proj_thunks(0):
                        th()

                if variant in ("dmaonly", "noattn"):
                    for pg in range(NPG):
                        nc.vector.memset(attnT[:, pg, :], 0.01)
                    if variant == "noattn":
                        for pg in range(1, NPG):
                            for th in proj_thunks(pg):
                                th()
                        st = {}
                        for c in range(NCH):
                            for th in outproj_chunk_thunks(c, st):
                                th()
                    else:
                        for c in range(NCH):
                            o_big = mp.tile(
                                [128, 4, E], BF, tag="osb", bufs=2, name="o_big"
                            )
                            nc.vector.memset(o_big, 0.0)
                            nc.gpsimd.dma_start(
                                out_d.ap()[512 * c : 512 * (c + 1), :].rearrange(
                                    "(r p) e -> p r e", p=128
                                ),
                                o_big,
                            )
                else:
                    for pg in range(NPG):
                        if pg < NPG - 1 and variant != "noproj":
                            feeds = deque(proj_thunks(pg + 1))
                        else:
                            feeds = deque()
                        if pg == NPG - 1:
                            op_state = {}

                            def release(c, feeds=feeds, op_state=op_state):
                                feeds.extend(outproj_chunk_thunks(c, op_state))

                            attention(pg, feeds, on_chunk_done=release)
                        else:
                            attention(pg, feeds)
                        while feeds:
                            feeds.popleft()()

    nc.compile()
    return nc


def _causal_mask_ref():
    return np.where(
        np.arange(T)[:, None] >= np.arange(T)[None, :], np.float32(0.0), np.float32(-1e9)
    ).astype(np.float32)


def _diag_patterns():
    # pattern[j, s, q] = 1.0 if (128*j + s) <= q else 0.0   (q in 0..511)
    j = np.arange(4)[:, None, None]
    s = np.arange(128)[None, :, None]
    q = np.arange(512)[None, None, :]
    return ((128 * j + s) <= q).astype(BF16)


def _prep_in_maps(query, key, value, attn_mask, wq, bq, wk, bk, wv, bv, wo, causal):
    common = {}
    if causal:
        common["dmask"] = np.ascontiguousarray(_diag_patterns())
    else:
        common["emaskT"] = np.exp(attn_mask.astype(np.float64).T).astype(BF16)
    in_maps = []
    xT = {}
    for b in range(B):
        xT[b] = {
            "xq": np.ascontiguousarray(query[:, b, :].T).astype(BF16),
            "xk": np.ascontiguousarray(key[:, b, :].T).astype(BF16),
            "xv": np.ascontiguousarray(value[:, b, :].T).astype(BF16),
        }
    for c in range(NCORES):
        b, hh = divmod(c, 2)
        sl = slice(DC * hh, DC * (hh + 1))
        m = dict(common)
        m.update(xT[b])
        m["wqT"] = np.ascontiguousarray(wq[sl, :].T).astype(BF16)
        m["wkT"] = np.ascontiguousarray(wk[sl, :].T).astype(BF16)
        m["wvT"] = np.ascontiguousarray(wv[sl, :].T).astype(BF16)
        m["woT"] = np.ascontiguousarray(wo[:, sl].T).astype(BF16)
        m["bq"] = np.ascontiguousarray(bq[sl].astype(np.float32).reshape(NPG, 128).T)
        m["bk"] = np.ascontiguousarray(bk[sl].astype(np.float32).reshape(NPG, 128).T)
        m["bv"] = np.ascontiguousarray(bv[sl].astype(np.float32).reshape(NPG, 128).T)
        in_maps.append(m)
    return in_maps


def _postprocess(results, bo):
    outs = []
    for b in range(B):
        acc = results[2 * b]["out"].astype(np.float32) + results[2 * b + 1][
            "out"
        ].astype(np.float32)
        outs.append(acc)
    out = np.stack(outs, axis=1) + bo[None, None, :]
    return np.ascontiguousarray(out.astype(np.float32))


def kernel(query, key, value, attn_mask, wq, bq, wk, bk, wv, bv, wo, bo):
    assert query.shape == (T, B, E), query.shape
    causal = bool(np.array_equal(attn_mask, _causal_mask_ref()))
    if causal not in _CACHE:
        _CACHE[causal] = _build(causal)
    nc = _CACHE[causal]
    in_maps = _prep_in_maps(
        query, key, value, attn_mask, wq, bq, wk, bk, wv, bv, wo, causal
    )
    res = run_bass_kernel_spmd(nc, in_maps, core_ids=list(range(NCORES)))
    return _postprocess(res.results, np.asarray(bo, dtype=np.float32))


# revision 15
# speedup vs baseline: 1.1245x; 1.0399x over previous
"""Trainium2 Bass kernel for nn_CacheAttention (16-head causal MHA, T=2048 B=4 E=1024).

Sharding v2: core = (batch, head-half).  8 cores = 4 batches x 2 halves; each
core owns ONE batch and 8 heads (512 projection columns, processed as 4
partition-groups of 128 rows = 2 heads each).  vs v1 (2 heads x 4 batches per
core) this cuts per-core HBM traffic from 64 MB to 20 MB: q/k/v activations
for the core's batch are loaded once per rep and reused by all 4 groups.

Perf structure: the attention inner loop is ACT(exp)-bound, and the PE queue
is in-order, so every non-attention matmul (projections of the NEXT
partition-group, out-projection once the last group's chunk is normalized) is
emitted as fine-grained "feed" thunks BETWEEN attention instructions.  This
keeps the PE stream continuously busy, which both hides the work and holds
the PE at its top p-state (TRN2 DVFS: 2.4 GHz only after 3 us of
uninterrupted execution).  PV matmuls run one jp behind their exp (softmax
probabilities) so the PE never waits on ACT.  Score tiles are [128,1024]
PSUM (two s-tiles per exp); softmax sums ride along as a ones-column in V;
the causal path skips fully-masked tiles and uses 4 cached diagonal patterns.

Host sums each batch's two partial outputs and adds the output bias.
"""

import sys

if "/opt/trn_rl_repo" not in sys.path:
    sys.path.insert(0, "/opt/trn_rl_repo")

from collections import deque

import numpy as np
import ml_dtypes

import concourse.mybir as mybir
import concourse.tile as tile
from concourse import bacc
from concourse.bass_utils import run_bass_kernel_spmd
from concourse.masks import make_identity

BF16 = ml_dtypes.bfloat16
F32 = mybir.dt.float32
BF = mybir.dt.bfloat16

T, B, E = 2048, 4, 1024
H, D = 16, 64
NCORES = 8
HPC = 2                    # heads per partition-group
DC = 512                   # head-dim columns per core (8 heads)
NPG = DC // 128            # partition groups per core = 4
KT = E // 128              # E contraction tiles = 8
NCH = T // 512             # q chunks per (b,h) pair = 4
NST = T // 128             # s tiles = 16
SCALE = float(D) ** -0.5

_CACHE = {}


def _build(causal: bool, reps: int = 1, variant: str = "base"):
    # variant: base | dmaonly | noattn (attention replaced by memset) |
    #          noproj (projections replaced by memset) | nofeed (no
    #          interleaving: proj/outproj emitted flat)
    nc = bacc.Bacc("TRN2", target_bir_lowering=False, debug=False, num_devices=NCORES)

    xq_d = nc.dram_tensor("xq", [E, T], BF, kind="ExternalInput")
    xk_d = nc.dram_tensor("xk", [E, T], BF, kind="ExternalInput")
    xv_d = nc.dram_tensor("xv", [E, T], BF, kind="ExternalInput")
    wqT_d = nc.dram_tensor("wqT", [E, DC], BF, kind="ExternalInput")
    wkT_d = nc.dram_tensor("wkT", [E, DC], BF, kind="ExternalInput")
    wvT_d = nc.dram_tensor("wvT", [E, DC], BF, kind="ExternalInput")
    woT_d = nc.dram_tensor("woT", [DC, E], BF, kind="ExternalInput")
    bq_d = nc.dram_tensor("bq", [128, NPG], F32, kind="ExternalInput")
    bk_d = nc.dram_tensor("bk", [128, NPG], F32, kind="ExternalInput")
    bv_d = nc.dram_tensor("bv", [128, NPG], F32, kind="ExternalInput")
    if causal:
        dm_d = nc.dram_tensor("dmask", [4, 128, 512], BF, kind="ExternalInput")
    else:
        em_d = nc.dram_tensor("emaskT", [T, T], BF, kind="ExternalInput")
    out_d = nc.dram_tensor("out", [T, E], BF, kind="ExternalOutput")

    Exp = mybir.ActivationFunctionType.Exp
    add = mybir.AluOpType.add
    mult = mybir.AluOpType.mult

    with tile.TileContext(nc) as tc:
        with (
            tc.tile_pool(name="wp", bufs=1) as wp,
            tc.tile_pool(name="mp", bufs=1) as mp,
            tc.tile_pool(name="ps", bufs=2, space="PSUM") as ps,
        ):
            # ---- constants / weights (persistent) ----
            wq_sb = wp.tile([128, KT, DC], BF, tag="wq")
            wk_sb = wp.tile([128, KT, DC], BF, tag="wk")
            wv_sb = wp.tile([128, KT, DC], BF, tag="wv")
            for w_sb, w_d in ((wq_sb, wqT_d), (wk_sb, wkT_d), (wv_sb, wvT_d)):
                nc.sync.dma_start(w_sb, w_d.ap().rearrange("(k p) d -> p k d", p=128))
            wo_sb = wp.tile([128, NPG, E], BF, tag="wo")
            nc.sync.dma_start(wo_sb, woT_d.ap().rearrange("(g p) e -> p g e", p=128))
            bq_sb = wp.tile([128, NPG], F32, tag="bq")
            nc.sync.dma_start(bq_sb, bq_d.ap())
            bk_sb = wp.tile([128, NPG], F32, tag="bk")
            nc.sync.dma_start(bk_sb, bk_d.ap())
            bv_sb = wp.tile([128, NPG], F32, tag="bv")
            nc.sync.dma_start(bv_sb, bv_d.ap())
            ident = wp.tile([128, 128], BF, tag="ident")
            make_identity(nc, ident)
            ones64 = wp.tile([1, 64], BF, tag="ones64")
            nc.vector.memset(ones64, 1.0)
            if causal:
                dm_sb = wp.tile([128, 4 * 512], BF, tag="dm")
                nc.sync.dma_start(
                    dm_sb.rearrange("p (j q) -> p j q", q=512),
                    dm_d.ap().rearrange("j p q -> p j q"),
                )

            # rep-persistent activation tiles
            qT_all = wp.tile([128, NPG, T], BF, tag="qT")
            kT_all = wp.tile([128, NPG, T], BF, tag="kT")
            v_nat = wp.tile([128, NPG, NST * 130], BF, tag="vnat")
            attnT = wp.tile([128, NPG, T], BF, tag="attnT")

            def proj_thunks(pg):
                """Feed thunks computing qT/kT/v_nat for group pg: 3 x
                (dma + 2 psum-pair passes over 8 k-tiles + bias) + v
                transposes.  Each thunk is a small, stall-free PE burst."""
                thunks = []
                plan = (
                    (xq_d, wq_sb, bq_sb, SCALE, qT_all),
                    (xk_d, wk_sb, bk_sb, 1.0, kT_all),
                    (xv_d, wv_sb, bv_sb, 1.0, None),
                )
                xins = {}
                vTt = None

                def dma_xin(i):
                    def th():
                        src_d = plan[i][0]
                        xt = mp.tile([128, KT, T], BF, tag="xin", bufs=2, name="xin")
                        src_v = src_d.ap().rearrange("(k p) r -> p k r", p=128)
                        kk = KT // 2
                        for half in range(2):
                            nc.sync.dma_start(
                                xt[:, half * kk : (half + 1) * kk, :],
                                src_v[:, half * kk : (half + 1) * kk, :],
                            )
                        xins[i] = xt
                    return th

                def mk_group(i, g):
                    # one psum pair accumulating two 512-col chunks over all k
                    st = {}

                    def alloc():
                        st["pps"] = [
                            ps.tile([128, 512], F32, tag="mm", name=f"pp{j}")
                            for j in range(2)
                        ]

                    def mm(k):
                        def th():
                            if "pps" not in st:
                                alloc()
                            w_sb = plan[i][1]
                            for j in range(2):
                                n = 2 * g + j
                                nc.tensor.matmul(
                                    st["pps"][j],
                                    w_sb[:, k, 128 * pg : 128 * (pg + 1)],
                                    xins[i][:, k, 512 * n : 512 * (n + 1)],
                                    start=(k == 0),
                                    stop=(k == KT - 1),
                                )
                        return th

                    def bias():
                        nonlocal vTt
                        _, _, b_sb, scale, dst = plan[i]
                        if dst is None and vTt is None:
                            vTt = mp.tile([128, T], BF, tag="vTt", bufs=1)
                        for j in range(2):
                            n = 2 * g + j
                            dst_ap = (
                                vTt[:, 512 * n : 512 * (n + 1)]
                                if dst is None
                                else dst[:, pg, 512 * n : 512 * (n + 1)]
                            )
                            nc.vector.tensor_scalar(
                                dst_ap,
                                st["pps"][j],
                                b_sb[:, pg : pg + 1],
                                scale,
                                add,
                                mult,
                            )
                    return [mm(k) for k in range(KT)] + [bias]

                thunks.append(dma_xin(0))
                thunks.append(dma_xin(1))
                thunks.extend(mk_group(0, 0))
                thunks.extend(mk_group(0, 1))
                thunks.extend(mk_group(1, 0))
                thunks.append(dma_xin(2))
                thunks.extend(mk_group(1, 1))
                thunks.extend(mk_group(2, 0))
                thunks.extend(mk_group(2, 1))

                def ones():
                    vv = v_nat[:, pg, :].rearrange("p (r c) -> p r c", c=65)
                    nc.vector.memset(vv[:, :, 64], 1.0)
                thunks.append(ones)

                def transp(j):
                    def th():
                        pt = ps.tile([128, 128], BF, tag="mm", name="pt")
                        nc.tensor.transpose(pt, vTt[:, 128 * j : 128 * (j + 1)], ident)
                        for h in range(HPC):
                            nc.vector.tensor_copy(
                                v_nat[:, pg, 130 * j + 65 * h : 130 * j + 65 * h + 64],
                                pt[:, 64 * h : 64 * h + 64],
                            )
                    return th

                thunks.extend(transp(j) for j in range(NST))
                return thunks

            def outproj_chunk_thunks(c, st):
                # out rows 512c..512c+511; contraction over all 4 groups
                thunks = []

                def alloc():
                    st["o_big"] = mp.tile([128, 4, E], BF, tag="osb", bufs=2, name="o_big")

                def mk(r4, n):
                    def th():
                        if "o_big" not in st:
                            alloc()
                        r = 4 * c + r4
                        o_ps = ps.tile([128, 512], F32, tag="mm", name="o_ps")
                        for pg in range(NPG):
                            nc.tensor.matmul(
                                o_ps,
                                attnT[:, pg, 128 * r : 128 * (r + 1)],
                                wo_sb[:, pg, 512 * n : 512 * (n + 1)],
                                start=(pg == 0),
                                stop=(pg == NPG - 1),
                            )
                        nc.vector.tensor_copy(o_big_slice(st, r4, n), o_ps)
                    return th

                def o_big_slice(st, r4, n):
                    return st["o_big"][:, r4, 512 * n : 512 * (n + 1)]

                for r4 in range(4):
                    for n in range(E // 512):
                        thunks.append(mk(r4, n))

                def store():
                    nc.gpsimd.dma_start(
                        out_d.ap()[512 * c : 512 * (c + 1), :].rearrange(
                            "(r p) e -> p r e", p=128
                        ),
                        st.pop("o_big"),
                    )
                thunks.append(store)
                return thunks

            def attention(pg, feeds, on_chunk_done=None):
                """Attention for group pg (2 heads), feeding `feeds` thunks
                into the PE stream.  PV runs one jp behind its exp.
                on_chunk_done(c) may extend `feeds` (out-proj release)."""
                def feed(k):
                    for _ in range(k):
                        if feeds:
                            feeds.popleft()()

                for c in range(NCH):
                    n_s = 4 * (c + 1) if causal else NST
                    a_ps = [
                        ps.tile([65, 512], F32, tag=f"at{h}", bufs=1, name=f"a_ps{h}")
                        for h in range(HPC)
                    ]
                    prev = None

                    def emit_pv(item, n_s=n_s):
                        # pTs keyed (h, dj); each [128, 512]
                        pTs, jpp = item
                        for h in range(HPC):
                            for dj in range(2):
                                nc.tensor.matmul(
                                    a_ps[h],
                                    v_nat[
                                        :,
                                        pg,
                                        130 * (2 * jpp + dj) + 65 * h : 130 * (2 * jpp + dj) + 65 * (h + 1),
                                    ],
                                    pTs[h][dj],
                                    start=(jpp == 0 and dj == 0),
                                    stop=(jpp == n_s // 2 - 1 and dj == 1),
                                )

                    for jp in range(n_s // 2):
                        j0 = 2 * jp
                        # one sc tag, bufs=4, allocated in the SAME (dj-major)
                        # order the ACTs free them: next jp's first QK pair
                        # only waits for this jp's first exp, hiding the
                        # PE<->ACT semaphore round-trip entirely.
                        sc = {}
                        for dj in range(2):
                            for h in range(HPC):
                                sc[(h, dj)] = ps.tile(
                                    [128, 512], F32, tag="sc", bufs=4, name="sc"
                                )
                        for dj in range(2):
                            j = j0 + dj
                            for h in range(HPC):
                                hs = 64 * h
                                nc.tensor.matmul(
                                    sc[(h, dj)],
                                    kT_all[hs : hs + 64, pg, 128 * j : 128 * (j + 1)],
                                    qT_all[hs : hs + 64, pg, 512 * c : 512 * (c + 1)],
                                    start=True,
                                    stop=True,
                                )
                        feed(1)
                        em0 = None
                        if not causal:
                            em0 = mp.tile([128, 1024], BF, tag="em", bufs=4, name="em")
                            for dj in range(2):
                                nc.sync.dma_start(
                                    em0[:, 512 * dj : 512 * (dj + 1)],
                                    em_d.ap()[
                                        128 * (j0 + dj) : 128 * (j0 + dj + 1),
                                        512 * c : 512 * (c + 1),
                                    ],
                                )
                        elif j0 >= 4 * c:
                            em0 = dm_sb[:, 512 * (j0 - 4 * c) : 512 * (j0 - 4 * c) + 1024]
                        pTs = {}
                        for dj in range(2):
                            for h in range(HPC):
                                pT = mp.tile([128, 512], BF, tag="pT", bufs=8, name="pT")
                                nc.scalar.activation(pT, sc[(h, dj)], Exp)
                                if em0 is not None:
                                    pm = mp.tile(
                                        [128, 512], BF, tag="pm", bufs=4, name="pm"
                                    )
                                    nc.vector.tensor_tensor(
                                        pm, pT, em0[:, 512 * dj : 512 * (dj + 1)], mult
                                    )
                                    pT = pm
                                pTs.setdefault(h, {})[dj] = pT
                        if prev is not None:
                            emit_pv(prev)
                        feed(2)
                        prev = (pTs, jp)
                    if prev is not None:
                        emit_pv(prev)
                    # softmax normalization -> attnT.  The reciprocal row is
                    # broadcast across partitions with a K=1 PE matmul
                    # (ones[1,64].T @ rl[1,512]) -- GpSimd partition_broadcast
                    # costs ~microseconds of Q7 dispatch on this critical path.
                    for h in range(HPC):
                        hs = 64 * h
                        rl = mp.tile([1, 512], BF, tag="rl", bufs=2, name="rl")
                        nc.vector.tensor_copy(rl, a_ps[h][64:65, :])
                        den_b = ps.tile([64, 512], F32, tag="sc", bufs=4, name="den_b")
                        nc.tensor.matmul(den_b, ones64, rl, start=True, stop=True)
                        rlb = mp.tile([64, 512], BF, tag="rlb", bufs=2, name="rlb")
                        with nc.allow_low_precision(reason="softmax denom recip"):
                            nc.vector.reciprocal(rlb, den_b)
                        nc.vector.tensor_tensor(
                            attnT[hs : hs + 64, pg, 512 * c : 512 * (c + 1)],
                            a_ps[h][0:64, :],
                            rlb,
                            mult,
                        )
                    if on_chunk_done is not None:
                        on_chunk_done(c)

            def memset_group(pg):
                nc.vector.memset(qT_all[:, pg, :], 0.02)
                nc.vector.memset(kT_all[:, pg, :], 0.02)
                nc.vector.memset(v_nat[:, pg, :], 0.01)

            # ================= main schedule =================
            for rep in range(reps):
                if variant in ("dmaonly", "noproj"):
                    # still move the input bytes
                    for src_d in (xq_d, xk_d, xv_d):
                        xt = mp.tile([128, KT, T], BF, tag="xin", bufs=2, name="xin")
                        nc.sync.dma_start(
                            xt, src_d.ap().rearrange("(k p) r -> p k r", p=128)
                        )
                    for pg in range(NPG):
                        memset_group(pg)
                else:
                    for th in proj_thunks(0):
                        th()

                if variant in ("dmaonly", "noattn"):
                    for pg in range(NPG):
                        nc.vector.memset(attnT[:, pg, :], 0.01)
                    if variant == "noattn":
                        for pg in range(1, NPG):
                            for th in proj_thunks(pg):
                                th()
                        st = {}
                        for c in range(NCH):
                            for th in outproj_chunk_thunks(c, st):
                                th()
                    else:
                        for c in range(NCH):
                            o_big = mp.tile(
                                [128, 4, E], BF, tag="osb", bufs=2, name="o_big"
                            )
                            nc.vector.memset(o_big, 0.0)
                            nc.gpsimd.dma_start(
                                out_d.ap()[512 * c : 512 * (c + 1), :].rearrange(
                                    "(r p) e -> p r e", p=128
                                ),
                                o_big,
                            )
                else:
                    for pg in range(NPG):
                        if pg < NPG - 1 and variant != "noproj":
                            feeds = deque(proj_thunks(pg + 1))
                        else:
                            feeds = deque()
                        if pg == NPG - 1:
                            op_state = {}

                            def release(c, feeds=feeds, op_state=op_state):
                                feeds.extend(outproj_chunk_thunks(c, op_state))

                            attention(pg, feeds, on_chunk_done=release)
                        else:
                            attention(pg, feeds)
                        while feeds:
                            feeds.popleft()()

    nc.compile()
    return nc


def _causal_mask_ref():
    return np.where(
        np.arange(T)[:, None] >= np.arange(T)[None, :], np.float32(0.0), np.float32(-1e9)
    ).astype(np.float32)


def _diag_patterns():
    # pattern[j, s, q] = 1.0 if (128*j + s) <= q else 0.0   (q in 0..511)
    j = np.arange(4)[:, None, None]
    s = np.arange(128)[None, :, None]
    q = np.arange(512)[None, None, :]
    return ((128 * j + s) <= q).astype(BF16)


def _prep_in_maps(query, key, value, attn_mask, wq, bq, wk, bk, wv, bv, wo, causal):
    common = {}
    if causal:
        common["dmask"] = np.ascontiguousarray(_diag_patterns())
    else:
        common["emaskT"] = np.exp(attn_mask.astype(np.float64).T).astype(BF16)
    in_maps = []
    xT = {}
    for b in range(B):
        xT[b] = {
            "xq": np.ascontiguousarray(query[:, b, :].T).astype(BF16),
            "xk": np.ascontiguousarray(key[:, b, :].T).astype(BF16),
            "xv": np.ascontiguousarray(value[:, b, :].T).astype(BF16),
        }
    for c in range(NCORES):
        b, hh = divmod(c, 2)
        sl = slice(DC * hh, DC * (hh + 1))
        m = dict(common)
        m.update(xT[b])
        m["wqT"] = np.ascontiguousarray(wq[sl, :].T).astype(BF16)
        m["wkT"] = np.ascontiguousarray(wk[sl, :].T).astype(BF16)
        m["wvT"] = np.ascontiguousarray(wv[sl, :].T).astype(BF16)
        m["woT"] = np.ascontiguousarray(wo[:, sl].T).astype(BF16)
        m["bq"] = np.ascontiguousarray(bq[sl].astype(np.float32).reshape(NPG, 128).T)
        m["bk"] = np.ascontiguousarray(bk[sl].astype(np.float32).reshape(NPG, 128).T)
        m["bv"] = np.ascontiguousarray(bv[sl].astype(np.float32).reshape(NPG, 128).T)
        in_maps.append(m)
    return in_maps


def _postprocess(results, bo):
    outs = []
    for b in range(B):
        acc = results[2 * b]["out"].astype(np.float32) + results[2 * b + 1][
            "out"
        ].astype(np.float32)
        outs.append(acc)
    out = np.stack(outs, axis=1) + bo[None, None, :]
    return np.ascontiguousarray(out.astype(np.float32))


def kernel(query, key, value, attn_mask, wq, bq, wk, bk, wv, bv, wo, bo):
    assert query.shape == (T, B, E), query.shape
    causal = bool(np.array_equal(attn_mask, _causal_mask_ref()))
    if causal not in _CACHE:
        _CACHE[causal] = _build(causal)
    nc = _CACHE[causal]
    in_maps = _prep_in_maps(
        query, key, value, attn_mask, wq, bq, wk, bk, wv, bv, wo, causal
    )
    res = run_bass_kernel_spmd(nc, in_maps, core_ids=list(range(NCORES)))
    return _postprocess(res.results, np.asarray(bo, dtype=np.float32))
